# revision 7
# baseline (speedup 1.0000x reference)
"""Trainium2 Bass kernel for CubicModel: out = feats(feats(x)@W0.T+b0)@W1.T+b1
where feats(z) = [z, triu(z_i z_j), z^3].

Strategy (8 NeuronCores, tensor-parallel over the 132352-dim feature axis,
batch-chunk pipelining to overlap the inter-layer collectives):

  * Merged-pair feature generation: difference classes d0=2t+1 (odd) and
    d1=2t+2 (even) share the same parity-split Y2 slot j=r0//2 and the same
    e-window (e = r0+d0 = r1+d1), so ONE [128, B] tensor_tensor op per k-tile
    generates both 64-row classes (DVE cost is partition-count independent).
    Specials: t=127 pairs the x_i^2 loops (r=0) with class d=255 (r=1) via a
    combo e-window; t=128 linear+cubic; t=129 the 32-row d=256 class + pad.
  * Batch split into two 256-col chunks X/Y. Layer-0 processes chunk X
    staggered DELTA0 k-tiles ahead of Y over a shared weight-tile ring
    (weights DMA'd once); X finishes ~40 us early so its ReduceScatter +
    bias + AllGather + L1-operand gathers hide under Y's remaining compute.
  * L1 operands (Y2h shift array, e-windows, d256 rows) are fetched straight
    from the AllGather output with multi-column indirect DMA gathers using
    per-core index arrays -- no PE/ACT work between the layers.
  * Engine roles: SP = weight/const DMA, ACT = psum evac + chain loads/
    stores + bias, Pool = collectives + gathers, DVE = feature gen ops,
    PE = matmul.  Per-chunk ReduceScatter outputs get bias on ACT; final
    layer-1 ReduceScatter per chunk writes the [32, B] output slice.
"""

import sys

sys.path.insert(0, "/opt/trn_rl_repo")

import numpy as np

N_CORES = 8
D = 512          # d_in == hidden
B = 512          # batch
H = 512          # hidden
DOUT = 256
KT = 130                    # k-tiles per core
QUAD_BASE = D
CUBIC_BASE = D + (D * D + D) // 2    # 131840

CH = 256                    # batch chunk (X = cols 0:256, Y = 256:512)
DELTA0 = 96                 # layer-0 stagger (k-tiles X runs ahead of Y)
WBUFS = 27                  # weight ring slots (window = 4*WBUFS k-tiles;
                            # must exceed DELTA0 by >=2 allocs of DMA lead)
NA0 = 33                    # layer-0 weight chunk-allocs (4 k-tiles each)
NA1 = 17                    # layer-1 weight chunk-allocs (8 k-tiles each)

# processing order: y2 slot j ascending within each 32-block, then specials
ORDER = ([31 - i for i in range(32)] + [63 - i for i in range(32)]
         + [95 - i for i in range(32)] + [126 - i for i in range(31)]
         + [127, 128, 129])


def _tile_params(t):
    """Merged tile t in [0,126]: (j, slot, r0, r1, d0, d1)."""
    d0 = 2 * t + 1
    d1 = 2 * t + 2
    r0 = (-d0) % 64
    return r0 // 2, t // 32, r0, r0 - 1, d0, d1


def _quad_idx(i, j):
    lo, hi = np.minimum(i, j), np.maximum(i, j)
    return QUAD_BASE + lo * D - lo * (lo - 1) // 2 + (hi - lo)


def _refk_for_tile(c, t):
    """Feature indices [128] for tile t on core c (-1 = zero pad)."""
    p = np.arange(64)
    out = np.full(128, -1, dtype=np.int64)
    if t <= 126:
        _, _, r0, r1, d0, d1 = _tile_params(t)
        i1 = (64 * c + r1 + p) % D
        out[0:64] = _quad_idx(i1, (i1 + d1) % D)
        i0 = (64 * c + r0 + p) % D
        out[64:128] = _quad_idx(i0, (i0 + d0) % D)
    elif t == 127:
        i = (64 * c + p) % D
        out[0:64] = _quad_idx(i, i)
        i0 = (64 * c + 1 + p) % D
        out[64:128] = _quad_idx(i0, (i0 + 255) % D)
    elif t == 128:
        out[0:64] = (64 * c + p) % D
        out[64:128] = CUBIC_BASE + (64 * c + p) % D
    else:
        k = np.arange(32)
        a = 32 * c + k
        out[0:32] = _quad_idx(a, (a + 256) % D)
    return out


def _w_rows(c, WT16, m):
    """[KT, 128, m] fp16 weight rows in processing order, pad rows zero."""
    rows = np.zeros((KT, 128, m), dtype=np.float16)
    for s in range(KT):
        refk = _refk_for_tile(c, ORDER[s])
        msk = refk >= 0
        rows[s, msk] = WT16[refk[msk]]
    return rows


def _gen_arrays(c, xT16):
    """y2 [128,32,B], ew [128,6,B], d2 [32,2,B] from a [D,B] fp16 array."""
    frame = xT16[(64 * c + np.arange(128)) % D]
    y2 = np.zeros((128, 32, B), dtype=np.float16)
    for r in range(64):
        y2[64 * (r % 2):64 * (r % 2) + 64, r // 2] = frame[r:r + 64]
    ew = np.zeros((128, 5, B), dtype=np.float16)
    p = np.arange(64)
    for sl in range(4):
        rows = (64 * c + 64 * (sl + 1) + p) % D
        ew[0:64, sl] = xT16[rows]
        ew[64:128, sl] = xT16[rows]
    ew[0:64, 4] = xT16[(64 * c + p) % D]
    ew[64:128, 4] = xT16[(64 * c + 256 + p) % D]
    d2 = np.zeros((32, 2, B), dtype=np.float16)
    k = np.arange(32)
    d2[:, 0] = xT16[(32 * c + k) % D]
    d2[:, 1] = xT16[(32 * c + 256 + k) % D]
    return y2, ew, d2


def _idx_arrays(c):
    """ifr [128, 4]: cols 0-2 = rotated-frame rows 128f+p; col 3 = packed d2
    rows (k<32: in0 rows, 32<=k<64: in1 rows, rest dummy)."""
    ifr = np.zeros((128, 4), dtype=np.int32)
    p = np.arange(128)
    for f in range(3):
        ifr[:, f] = (64 * c + 128 * f + p) % D
    k = np.arange(32)
    ifr[0:32, 3] = (32 * c + k) % D
    ifr[32:64, 3] = (32 * c + 256 + k) % D
    return ifr


def _prep_core_inputs(c, x16T, W0T, W1T, b0, b1):
    w0rows = _w_rows(c, W0T, H)                       # [130, 128, 512]
    w0pad = np.zeros((NA0 * 4, 128, H), dtype=np.float16)
    w0pad[:KT] = w0rows
    w0td = np.ascontiguousarray(
        w0pad.reshape(NA0, 4, 128, H).transpose(0, 2, 1, 3))

    w1rows = _w_rows(c, W1T, DOUT)                    # [130, 128, 256]
    w1pad = np.zeros((NA1 * 8, 128, DOUT), dtype=np.float16)
    w1pad[:KT] = w1rows
    # [17, 128, 4, 512] with [a, p, kk, 256*j:...] = w1rows[8a+2kk+j, p]
    w1td = np.ascontiguousarray(
        w1pad.reshape(NA1, 4, 2, 128, DOUT).transpose(0, 3, 1, 2, 4)
        .reshape(NA1, 128, 4, 2 * DOUT))

    y2, ew, d2 = _gen_arrays(c, x16T)
    ifr = _idx_arrays(c)

    return {
        "w0td": w0td,
        "w1td": w1td,
        "y2": y2,
        "ew": ew,
        "d2": d2,
        "ifr": ifr,
        "b0p": b0[64 * c:64 * c + 64].astype(np.float32).reshape(64, 1),
        "b1p": b1[32 * c:32 * c + 32].astype(np.float32).reshape(32, 1),
    }


def _build_program(repeat=1):
    import concourse.mybir as mybir
    import concourse.tile as tile
    from concourse import bacc
    from concourse.bass import IndirectOffsetOnAxis, AP

    fp16 = mybir.dt.float16
    f32 = mybir.dt.float32
    i32 = mybir.dt.int32
    Copy = mybir.ActivationFunctionType.Copy
    Ident = mybir.ActivationFunctionType.Identity
    CORE_IDS = list(range(N_CORES))

    import contextlib

    nc = bacc.Bacc(None, target_bir_lowering=False, debug=False)
    with tile.TileContext(nc) as tc:
        with tc.tile_pool(name="dram", bufs=1, space="DRAM") as dram, \
             tc.tile_pool(name="const", bufs=1) as const, \
             tc.tile_pool(name="wpool", bufs=WBUFS) as wpool, \
             tc.tile_pool(name="fpool", bufs=6) as fpool, \
             tc.tile_pool(name="spool", bufs=2) as spool, \
             tc.tile_pool(name="gpool", bufs=2) as gpool, \
             tc.tile_pool(name="ps", bufs=8, space="PSUM") as ps:
            # ---- DRAM I/O ----
            w0td = dram.tile([NA0, 128, 4, H], fp16, kind="ExternalInput", name="w0td", uniquify=False)
            w1td = dram.tile([NA1, 128, 4, 2 * DOUT], fp16, kind="ExternalInput", name="w1td", uniquify=False)
            y2d = dram.tile([128, 32, B], fp16, kind="ExternalInput", name="y2", uniquify=False)
            ewd = dram.tile([128, 5, B], fp16, kind="ExternalInput", name="ew", uniquify=False)
            d2d = dram.tile([32, 2, B], fp16, kind="ExternalInput", name="d2", uniquify=False)
            ifrd = dram.tile([128, 4], i32, kind="ExternalInput", name="ifr", uniquify=False)
            b0pd = dram.tile([64, 1], f32, kind="ExternalInput", name="b0p", uniquify=False)
            b1pd = dram.tile([32, 1], f32, kind="ExternalInput", name="b1p", uniquify=False)
            outp = dram.tile([32, B], f32, kind="ExternalOutput", name="outp", uniquify=False)
            cc0_in = [dram.tile([H, CH], f32, name=f"cc0_in{q}", uniquify=False) for q in range(2)]
            rs0 = [dram.tile([64, CH], f32, name=f"rs0_{q}", uniquify=False) for q in range(2)]
            cc0h = [dram.tile([64, CH], fp16, name=f"cc0h{q}", uniquify=False) for q in range(2)]
            cc0_out = [dram.tile([H, CH], fp16, name=f"cc0_out{q}", uniquify=False, addr_space="Shared") for q in range(2)]
            hfd = [dram.tile([448, CH], fp16, name=f"hfd{q}", uniquify=False) for q in range(2)]
            cc1_in = [dram.tile([DOUT, CH], f32, name=f"cc1_in{q}", uniquify=False) for q in range(2)]
            rs1 = [dram.tile([32, CH], f32, name=f"rs1_{q}", uniquify=False) for q in range(2)]

            rep_cm = tc.For_i(0, repeat, 1) if repeat > 1 else contextlib.nullcontext()
            with rep_cm:
                # ---- startup constants, interleaved with first weight DMAs
                w_sbs = {}

                def load_w(layer, a):
                    w_sb = wpool.tile([128, 4, H], fp16, tag="w",
                                      name=f"wsb{layer}_{a}")
                    src = w0td if layer == 0 else w1td
                    nc.sync.dma_start(out=w_sb, in_=src[a])
                    w_sbs[(layer, a)] = w_sb

                load_w(0, 0)
                ew_sb = const.tile([128, 5, B], fp16)
                nc.sync.dma_start(out=ew_sb[:, 0:1, :], in_=ewd[:, 0:1, :])
                y2_sb = const.tile([128, 32, B], fp16)
                nc.sync.dma_start(out=y2_sb[:, 0:2, :], in_=y2d[:, 0:2, :])
                load_w(0, 1)
                nc.sync.dma_start(out=y2_sb[:, 2:8, :], in_=y2d[:, 2:8, :])
                nc.sync.dma_start(out=ew_sb[:, 1:5, :], in_=ewd[:, 1:5, :])
                load_w(0, 2)
                nc.sync.dma_start(out=y2_sb[:, 8:16, :], in_=y2d[:, 8:16, :])
                load_w(0, 3)
                nc.sync.dma_start(out=y2_sb[:, 16:24, :], in_=y2d[:, 16:24, :])
                load_w(0, 4)
                nc.sync.dma_start(out=y2_sb[:, 24:32, :], in_=y2d[:, 24:32, :])
                load_w(0, 5)
                d2_sb = const.tile([32, 2, B], fp16)
                nc.sync.dma_start(out=d2_sb, in_=d2d[:])
                ifr_sb = const.tile([128, 4], i32)
                nc.sync.dma_start(out=ifr_sb, in_=ifrd[:])
                b0p_sb = const.tile([64, 1], f32)
                nc.sync.dma_start(out=b0p_sb, in_=b0pd[:])
                b1p_sb = const.tile([32, 1], f32)
                nc.sync.dma_start(out=b1p_sb, in_=b1pd[:])

                def emit_gen(s, y2t, ewt, d2t, cs):
                    """Feature tile for processing position s -> ft [128, CH]."""
                    t = ORDER[s]
                    ft = fpool.tile([128, CH], fp16, tag="ft")
                    if t <= 126:
                        j, slot, *_ = _tile_params(t)
                        nc.vector.tensor_mul(ft, y2t[:, j, cs], ewt[:, slot, cs])
                    elif t == 127:
                        nc.vector.tensor_mul(ft, y2t[:, 0, cs], ewt[:, 4, cs])
                    elif t == 128:
                        sq = spool.tile([64, CH], fp16, tag="sq")
                        nc.vector.tensor_copy(ft[0:64, :], y2t[0:64, 0, cs])
                        nc.vector.tensor_mul(sq, y2t[0:64, 0, cs], y2t[0:64, 0, cs])
                        nc.vector.tensor_mul(ft[64:128, :], sq, y2t[0:64, 0, cs])
                    else:
                        nc.vector.tensor_mul(ft[0:32, :], d2t[0:32, 0, cs], d2t[0:32, 1, cs])
                        nc.vector.memset(ft[32:64, :], 0)
                        nc.vector.memset(ft[64:128, :], 0)
                    return ft

                def emit_mm(layer, s, ft, psl):
                    if layer == 0:
                        a, kk, off = s // 4, s % 4, 0
                    else:
                        a, kk, off = s // 8, (s % 8) // 2, 256 * (s % 2)
                    w_sb = w_sbs[(layer, a)]
                    for h, pst in enumerate(psl):
                        nc.tensor.matmul(pst[:, 0:CH],
                                         w_sb[:, kk, off + 128 * h:off + 128 * h + 128],
                                         ft, start=(s == 0), stop=(s == KT - 1))

                def l0_chain(q, psl):
                    for h in range(4):
                        stg = spool.tile([128, CH], f32, tag="evac", bufs=4)
                        nc.scalar.activation(stg, psl[h][:, 0:CH], Copy)
                        nc.scalar.dma_start(out=cc0_in[q][128 * h:128 * h + 128, :], in_=stg)
                    nc.gpsimd.collective_compute(
                        "ReduceScatter", mybir.AluOpType.add,
                        replica_groups=[CORE_IDS], ins=[cc0_in[q][:]], outs=[rs0[q][:]])
                    rsp = spool.tile([64, CH], f32, tag="rsp")
                    nc.scalar.dma_start(out=rsp, in_=rs0[q][:])
                    rsb = spool.tile([64, CH], fp16, tag="rsb")
                    nc.scalar.activation(rsb, rsp, Ident, bias=b0p_sb[:, 0:1])
                    nc.scalar.dma_start(out=cc0h[q][:], in_=rsb)
                    nc.gpsimd.collective_compute(
                        "AllGather", mybir.AluOpType.bypass,
                        replica_groups=[CORE_IDS], ins=[cc0h[q][:]], outs=[cc0_out[q][:]])
                    # rotated-frame gathers (single-col, HW-proven) -> DRAM hfd
                    hf = hfd[q]
                    for f in range(4):
                        fr = spool.tile([128, CH], fp16, tag="fr", bufs=4,
                                        name=f"fr{q}_{f}")
                        nc.gpsimd.indirect_dma_start(
                            out=fr[:, :], out_offset=None, in_=cc0_out[q][:],
                            in_offset=IndirectOffsetOnAxis(ap=ifr_sb[:, f:f + 1], axis=0))
                        if f < 3:
                            nc.scalar.dma_start(out=hf[128 * f:128 * f + 128, :], in_=fr)
                        else:
                            nc.scalar.dma_start(out=hf[384:448, :], in_=fr[0:64, :])
                    # plain-slice window loads (dep-tracked)
                    ewh = gpool.tile([128, 5, CH], fp16, tag="ewh")
                    nc.scalar.dma_start(out=ewh[0:64, 0, :], in_=hf[64:128, :])
                    nc.scalar.dma_start(out=ewh[64:128, 0, :], in_=hf[64:128, :])
                    # custom overlapping-AP y2h loads; ordering vs hfd f0 store is
                    # guaranteed by the tracked ewh slot-0 loads just dispatched on
                    # the same engine queue (all y2h rows lie in hfd[0:128))
                    y2h = gpool.tile([128, 32, CH], fp16, tag="y2h")
                    hb = hf[:]
                    for piece in ((0, 8), (8, 32)):
                        j0, j1 = piece
                        for half in range(2):
                            src_ap = AP(hb.tensor, hb.offset + half * CH + 2 * j0 * CH,
                                        [[CH, 64], [2 * CH, j1 - j0], [1, CH]])
                            nc.scalar.dma_start(
                                out=y2h[64 * half:64 * half + 64, j0:j1, :], in_=src_ap)
                    for sl in range(1, 4):
                        nc.scalar.dma_start(out=ewh[0:64, sl, :], in_=hf[64 * (sl + 1):64 * (sl + 1) + 64, :])
                        nc.scalar.dma_start(out=ewh[64:128, sl, :], in_=hf[64 * (sl + 1):64 * (sl + 1) + 64, :])
                    nc.scalar.dma_start(out=ewh[0:64, 4, :], in_=hf[0:64, :])
                    nc.scalar.dma_start(out=ewh[64:128, 4, :], in_=hf[256:320, :])
                    d2h = gpool.tile([32, 2, CH], fp16, tag="d2h")
                    nc.scalar.dma_start(out=d2h[0:32, 0, :], in_=hf[384:416, :])
                    nc.scalar.dma_start(out=d2h[0:32, 1, :], in_=hf[416:448, :])
                    return y2h, ewh, d2h

                def l1_chain(q, psl):
                    for h in range(2):
                        stg = spool.tile([128, CH], f32, tag="evac1", bufs=4)
                        nc.scalar.activation(stg, psl[h][:, 0:CH], Copy)
                        nc.scalar.dma_start(out=cc1_in[q][128 * h:128 * h + 128, :], in_=stg)
                    nc.gpsimd.collective_compute(
                        "ReduceScatter", mybir.AluOpType.add,
                        replica_groups=[CORE_IDS], ins=[cc1_in[q][:]], outs=[rs1[q][:]])
                    rp = spool.tile([32, CH], f32, tag="rs1p")
                    nc.scalar.dma_start(out=rp, in_=rs1[q][:])
                    ob = spool.tile([32, CH], f32, tag="outb")
                    nc.scalar.activation(ob, rp, Ident, bias=b1p_sb[:, 0:1])
                    nc.scalar.dma_start(out=outp[:, q * CH:q * CH + CH], in_=ob)

                # ================= layer 0 (staggered X/Y) =================
                cslices = [np.s_[0:CH], np.s_[CH:2 * CH]]
                ps0 = [[ps.tile([128, B], f32, tag="ps", name=f"ps0_{q}_{h}")
                        for h in range(4)] for q in range(2)]
                g_handles = [None, None]
                for st in range(KT + DELTA0):
                    if st < KT:
                        s = st
                        if s % 4 == 0 and s // 4 >= 6:
                            load_w(0, s // 4)
                        ft = emit_gen(s, y2_sb, ew_sb, d2_sb, cslices[0])
                        emit_mm(0, s, ft, ps0[0])
                    if st >= DELTA0:
                        s = st - DELTA0
                        ft = emit_gen(s, y2_sb, ew_sb, d2_sb, cslices[1])
                        emit_mm(0, s, ft, ps0[1])
                    if st == KT - 1:
                        g_handles[0] = l0_chain(0, ps0[0])
                g_handles[1] = l0_chain(1, ps0[1])

                # ================= layer 1 (X pass then Y pass) =================
                for q in range(2):
                    y2h, ewh, d2h = g_handles[q]
                    psl = [ps.tile([128, B], f32, tag="ps", name=f"ps1_{q}_{h}")
                           for h in range(2)]
                    full = np.s_[0:CH]
                    for s in range(KT):
                        if q == 0 and s % 8 == 0:
                            load_w(1, s // 8)
                        ft = emit_gen(s, y2h, ewh, d2h, full)
                        emit_mm(1, s, ft, psl)
                    l1_chain(q, psl)
    nc.compile()
    return nc


_NC_CACHE = None


def build_in_maps(x, W0, b0, W1, b1):
    x16T = np.ascontiguousarray(x.T).astype(np.float16)          # [D, B]
    W0T = np.ascontiguousarray(W0.T).astype(np.float16)          # [K, H]
    W1T = np.ascontiguousarray(W1.T).astype(np.float16)          # [K, DOUT]
    return [_prep_core_inputs(c, x16T, W0T, W1T, b0, b1) for c in range(N_CORES)]


def kernel(x, W0, b0, W1, b1):
    global _NC_CACHE
    from concourse.bass_utils import run_bass_kernel_spmd

    in_maps = build_in_maps(x, W0, b0, W1, b1)
    if _NC_CACHE is None:
        _NC_CACHE = _build_program()
    res = run_bass_kernel_spmd(_NC_CACHE, in_maps, list(range(N_CORES)))
    outT = np.concatenate([res.results[c]["outp"] for c in range(N_CORES)], axis=0)
    return np.ascontiguousarray(outT.T.astype(np.float32))
